# revision 14
# baseline (speedup 1.0000x reference)
"""DecoderLSTM Trainium2 kernel v2 (8 NeuronCores, SPMD, no collectives).

Strategy (v2 — keeps the Tensor engine dense and warm):
  - LSTM recurrence replicated on all 8 cores (latency-bound, B=32).
  - Final projection sharded over vocab (VSH=6400 rows/core), computed
    TOKEN-STATIONARY: a [128-token x 512-hdim] block (4 steps of h1) is
    the PE stationary operand; Wf^T streams as the moving operand.
    Projection is interleaved per-step so it fills the PE-idle gaps the
    LSTM cell phases would otherwise leave (keeps HAM at K=8/8).
  - The x_t @ W_ih0[:, :E] (+ layer-0 bias) gate contribution does not
    depend on the recurrence OR the batch: ptab = emb @ W_ih0x + b0 is
    precomputed on the HOST ([V, 2048] bf16). The device just gathers
    128-token chunks of it (SWDGE indirect DMA) and injects them into
    each step's G0 PSUM accumulation with K=32 identity matmuls at
    tile_position rows.
  - Gates layout in PSUM: partition = 32*hq + b, free = g'*128 + hsub
    (gate order i,f,o,g), so elementwise ops use all 128 lanes.
  - All transposes are regular bf16 matmuls against an identity moving
    operand (faster than transpose-mode, and they keep HAM warm).
  - Output written as bf16 [NTOK, VSH] (halves the dominant HBM write);
    host reassembles/upcasts, and adds the vocab bias (usually zero).
"""

import os
import numpy as np
import ml_dtypes

K_ZBIAS = bool(int(os.environ.get("K_ZBIAS", "0")))

V, E, H, B, S = 50257, 512, 512, 32, 128
NC_ = 8
VSH = 6400                      # per-core padded vocab shard
VPAD = VSH * NC_                # 51200
NTOK = B * S                    # 4096 tokens, token = t*32 + b
NCHUNK = S // 4                 # 32 chunks of 4 steps = 128 tokens

# gate reorder: g' = (i, f, o, g) -> original pytorch order (i, f, g, o)
GPERM = (0, 1, 3, 2)

# vocab chunks for the projection moving operand (13 per 4-step group)
VCH = [(i * 512, min((i + 1) * 512, VSH)) for i in range((VSH + 511) // 512)]
PROJ_SCHED = {0: VCH[0:3], 1: VCH[3:6], 2: VCH[6:9], 3: VCH[9:13]}

_cache = {}


def _rearrange_w_cols(Wt):
    """Wt: [K, 4H] with original gate-column order (i,f,g,o) x H.
    Returns [K, 4H] with col' = hq*512 + g'*128 + hsub  mapping to
    original col = GPERM[g']*512 + hq*128 + hsub."""
    K = Wt.shape[0]
    w = Wt.reshape(K, 4, 4, 128)               # [K, g_orig, hq, hsub]
    out = np.empty((K, 4, 4, 128), Wt.dtype)   # [K, hq, g', hsub]
    for gp, go in enumerate(GPERM):
        out[:, :, gp, :] = w[:, go, :, :]
    return out.reshape(K, 4 * H)


def _g_layout_bias(bvec):
    """[4H] orig order -> [128, 512] G-layout tile (broadcast over b)."""
    r = _rearrange_w_cols(bvec.reshape(1, 4 * H))[0]   # col' order
    out = np.empty((128, 512), np.float32)
    for hq in range(4):
        out[32 * hq:32 * (hq + 1), :] = r[512 * hq:512 * (hq + 1)][None, :]
    return out


def _x2_layout(a):
    """[B, H] -> [128, 128] with partition 32*hq+b, free hsub."""
    return np.ascontiguousarray(
        a.reshape(B, 4, 128).transpose(1, 0, 2).reshape(128, 128))


def _hT_layout(a):
    """[B, H] -> [128, 128] with partition hsub, free hq*32+b."""
    return np.ascontiguousarray(
        a.reshape(B, 4, 128).transpose(2, 1, 0).reshape(128, 128))


def _build_program():
    import concourse.bass as bass
    import concourse.bacc as bacc
    import concourse.tile as tile
    from concourse import mybir

    f32 = mybir.dt.float32
    bf16 = mybir.dt.bfloat16
    i32 = mybir.dt.int32
    AF = mybir.ActivationFunctionType
    MUL = mybir.AluOpType.mult
    ADD = mybir.AluOpType.add

    nc = bacc.Bacc("TRN2", target_bir_lowering=False, debug=False,
                   enable_asserts=False, num_devices=NC_)

    d_seqG = nc.dram_tensor("seqG", [128, NCHUNK], i32, kind="ExternalInput").ap()
    d_ptab = nc.dram_tensor("ptab", [V, 4 * H], bf16, kind="ExternalInput").ap()
    d_wfe = nc.dram_tensor("wfe", [4, 128, 4 * H], bf16, kind="ExternalInput").ap()
    d_whh0 = nc.dram_tensor("whh0", [4, 128, 4 * H], bf16, kind="ExternalInput").ap()
    d_wih1 = nc.dram_tensor("wih1", [4, 128, 4 * H], bf16, kind="ExternalInput").ap()
    d_whh1 = nc.dram_tensor("whh1", [4, 128, 4 * H], bf16, kind="ExternalInput").ap()
    d_wfT = nc.dram_tensor("wfT", [4, 128, VSH], bf16, kind="ExternalInput").ap()
    d_identb = nc.dram_tensor("identb", [128, 128], bf16, kind="ExternalInput").ap()
    d_id4 = nc.dram_tensor("id4", [128, 32], bf16, kind="ExternalInput").ap()
    d_h0T = nc.dram_tensor("h0T", [128, 128], bf16, kind="ExternalInput").ap()
    d_h1T = nc.dram_tensor("h1T", [128, 128], bf16, kind="ExternalInput").ap()
    d_c0 = nc.dram_tensor("c0", [128, 128], f32, kind="ExternalInput").ap()
    d_c1 = nc.dram_tensor("c1", [128, 128], f32, kind="ExternalInput").ap()
    if not K_ZBIAS:
        d_b1g = nc.dram_tensor("b1g", [128, 512], bf16, kind="ExternalInput").ap()

    d_out = nc.dram_tensor("logits", [NTOK, VSH], bf16, kind="ExternalOutput").ap()

    with tile.TileContext(nc) as tc:
        consts = tc.alloc_tile_pool(name="consts", bufs=1)
        wpool = tc.alloc_tile_pool(name="weights", bufs=1)
        ppool = tc.alloc_tile_pool(name="pc", bufs=6)
        hpool = tc.alloc_tile_pool(name="hstate", bufs=3)
        cpool = tc.alloc_tile_pool(name="cstate", bufs=3)
        ewpool = tc.alloc_tile_pool(name="ew", bufs=3)
        bkp = tc.alloc_tile_pool(name="blk", bufs=2)
        stp = tc.alloc_tile_pool(name="stage", bufs=6)
        psg = tc.alloc_tile_pool(name="psg", bufs=3, space="PSUM")
        psx = tc.alloc_tile_pool(name="psx", bufs=5, space="PSUM")

        # ---- constants, initial state, first-needed weights ----
        identb = consts.tile([128, 128], bf16, tag="identb")
        nc.sync.dma_start(identb[:], d_identb[:])
        id4 = consts.tile([128, 32], bf16, tag="id4")
        nc.sync.dma_start(id4[:], d_id4[:])
        t_seqG = consts.tile([128, NCHUNK], i32, tag="seqG")
        nc.sync.dma_start(t_seqG[:], d_seqG[:])
        if not K_ZBIAS:
            t_b1g = consts.tile([128, 512], bf16, tag="b1g")
            nc.sync.dma_start(t_b1g[:], d_b1g[:])

        h0T = hpool.tile([128, 128], bf16, tag="h0T")
        nc.sync.dma_start(h0T[:], d_h0T[:])
        h1T = hpool.tile([128, 128], bf16, tag="h1T")
        nc.sync.dma_start(h1T[:], d_h1T[:])
        c0 = cpool.tile([128, 128], f32, tag="c0")
        nc.sync.dma_start(c0[:], d_c0[:])
        c1 = cpool.tile([128, 128], f32, tag="c1")
        nc.sync.dma_start(c1[:], d_c1[:])

        def load_w(dram, name):
            ts = []
            for k in range(4):
                t = wpool.tile([128, 4 * H], bf16, tag=f"{name}{k}")
                nc.sync.dma_start(t[:], dram[k])
                ts.append(t)
            return ts

        # ---- helpers ----
        def gather_chunk(c):
            """gather 128 rows of ptab -> P chunk [128 tok, 2048] bf16"""
            pc = ppool.tile([128, 4 * H], bf16, tag="pc")
            nc.gpsimd.indirect_dma_start(
                out=pc[:], out_offset=None, in_=d_ptab[:],
                in_offset=bass.IndirectOffsetOnAxis(ap=t_seqG[:, c:c + 1], axis=0),
            )
            return pc

        # weight loads, ordered by first use: whh0/whh1/wih1 at step 0,
        # wfe at step 1, wfT from step 4 (loaded vocab-chunk-interleaved
        # so the first projection group unblocks after ~0.5 MB)
        whh0 = load_w(d_whh0, "whh0")
        pcs = {c: gather_chunk(c) for c in range(6)}
        whh1 = load_w(d_whh1, "whh1")
        wih1 = load_w(d_wih1, "wih1")
        wfe = load_w(d_wfe, "wfe")
        wfT = []
        for k in range(4):
            t = wpool.tile([128, VSH], bf16, tag=f"wfT{k}")
            wfT.append(t)
        for (vlo, vhi) in VCH:
            for k in range(4):
                nc.sync.dma_start(wfT[k][:, vlo:vhi], d_wfT[k, :, vlo:vhi])

        def emit_group(G, hT, wts, first, last, split=False):
            """G += hT.T-strips @ wts (K=512 as 4 k-tiles x 4 col-strips).

            split=True emits the sigmoid-input columns [0:384] of every
            k-tile first, then the tanh columns [384:512], so the
            downstream sigmoid's subtile dependency clears ~400ns
            earlier than the full group."""
            ranges = [(0, 384), (384, 512)] if split else [(0, 512)]
            for lo, hi in ranges:
                for k in range(4):
                    lt = hT[:, 32 * k:32 * (k + 1)]
                    for cg in range(4):
                        nc.tensor.matmul(
                            G[32 * cg:32 * (cg + 1), lo:hi], lt,
                            wts[k][:, 512 * cg + lo:512 * cg + hi],
                            start=(first and k == 0), stop=(last and k == 3),
                            tile_position=(0, 32 * cg), skip_group_check=True)

        def inject_p(G, pc, s, first):
            """G[32cg+m, n] (+)= pc[32s+m, 512cg+n] via K=32 identity MMs."""
            for cg in range(4):
                nc.tensor.matmul(
                    G[32 * cg:32 * (cg + 1), :],
                    id4[32 * s:32 * (s + 1), :],
                    pc[32 * s:32 * (s + 1), 512 * cg:512 * (cg + 1)],
                    start=first, stop=False,
                    tile_position=(32 * s, 32 * cg), skip_group_check=True)

        def inject_full(G, src, first):
            """G (+)= src ([128,512]) via K=128 identity MM."""
            nc.tensor.matmul(G[:], identb[:], src[:], start=first, stop=False,
                             skip_group_check=True)

        def cell(G, cprev, ctag):
            sig = ewpool.tile([128, 384], f32, tag="sig")
            nc.scalar.activation(sig[:], G[:, 0:384], AF.Sigmoid)
            tg = ewpool.tile([128, 128], f32, tag="tg")
            nc.scalar.activation(tg[:], G[:, 384:512], AF.Tanh)
            m2 = ewpool.tile([128, 128], f32, tag="m2")
            nc.vector.tensor_tensor(m2[:], sig[:, 128:256], cprev[:], op=MUL)
            m1 = ewpool.tile([128, 128], f32, tag="m1")
            nc.vector.tensor_tensor(m1[:], sig[:, 0:128], tg[:], op=MUL)
            cn = cpool.tile([128, 128], f32, tag=ctag)
            nc.vector.tensor_tensor(cn[:], m1[:], m2[:], op=ADD)
            tc_ = ewpool.tile([128, 128], f32, tag="tc")
            nc.scalar.activation(tc_[:], cn[:], AF.Tanh)
            hx = ewpool.tile([128, 128], bf16, tag="hx")
            nc.vector.tensor_tensor(hx[:], sig[:, 256:384], tc_[:], op=MUL)
            return hx, cn

        def transpose_mm(hx):
            tp = psx.tile([128, 128], f32, tag="ps")
            nc.tensor.matmul(tp[:], hx[:], identb[:], start=True, stop=True)
            return tp

        def proj_group(bt, vlo, vhi, row0, eng="act"):
            n = vhi - vlo
            pj = psx.tile([128, 512], f32, tag="ps")
            for q in range(4):
                nc.tensor.matmul(pj[:, 0:n], bt[:, 128 * q:128 * (q + 1)],
                                 wfT[q][:, vlo:vhi],
                                 start=(q == 0), stop=(q == 3))
            st = stp.tile([128, 512], bf16, tag="st")
            if eng == "act":
                nc.scalar.copy(st[:, 0:n], pj[:, 0:n])
            else:
                nc.vector.tensor_copy(st[:, 0:n], pj[:, 0:n])
            nc.sync.dma_start(d_out[row0:row0 + 128, vlo:vhi], st[:, 0:n])

        # ---- main loop ----
        G0 = G1 = G0n = None
        blkT = blkT_prev = None

        for t in range(S):
            c, s = divmod(t, 4)
            if s == 0:
                blkT_prev, blkT = blkT, bkp.tile([128, 512], bf16, tag="blkT")
                if c + 6 < NCHUNK:
                    pcs[c + 6] = gather_chunk(c + 6)

            # (a) close G0(t): feed group (skipped at t=0: input_feed is 0)
            if t == 0:
                G0 = psg.tile([128, 512], f32, tag="G")
                inject_p(G0, pcs[0], 0, first=True)
                emit_group(G0, h0T, whh0, first=False, last=True, split=True)
            else:
                G0 = G0n
                emit_group(G0, h1T, wfe, first=False, last=True, split=True)

            # (b) cell0
            h0x, c0 = cell(G0, c0, "c0")

            # (c) prestart G1(t): h1prev part (+ bias) — fills cell0 gap
            G1 = psg.tile([128, 512], f32, tag="G")
            if not K_ZBIAS:
                inject_full(G1, t_b1g, first=True)
                emit_group(G1, h1T, whh1, first=False, last=False)
            else:
                emit_group(G1, h1T, whh1, first=True, last=False)

            # (d) projection fillers (chunk c-1); ACT copies queue after
            # cell0's activations so they never delay the cell chain
            if c >= 1:
                for (vlo, vhi) in PROJ_SCHED[s][:2]:
                    proj_group(blkT_prev, vlo, vhi, 128 * (c - 1))

            # (e) transpose h0
            tp0 = transpose_mm(h0x)
            h0T = hpool.tile([128, 128], bf16, tag="h0T")
            nc.vector.tensor_copy(h0T[:], tp0[:])

            # (f) close G1(t): h0 group
            emit_group(G1, h0T, wih1, first=False, last=True, split=True)

            # (g) cell1
            h1x, c1 = cell(G1, c1, "c1")

            # (h) prestart G0(t+1): P inject + h0prev — fills cell1 gap
            if t + 1 < S:
                cn_, sn = divmod(t + 1, 4)
                G0n = psg.tile([128, 512], f32, tag="G")
                inject_p(G0n, pcs[cn_], sn, first=True)
                emit_group(G0n, h0T, whh0, first=False, last=False)

            # (i) remaining projection fillers
            if c >= 1:
                rest = PROJ_SCHED[s][2:]
                for gi, (vlo, vhi) in enumerate(rest):
                    eng = "dve" if (s == 3 and gi == len(rest) - 1) else "act"
                    proj_group(blkT_prev, vlo, vhi, 128 * (c - 1), eng)

            # (j) transpose h1 -> h1T + blkT column
            tp1 = transpose_mm(h1x)
            h1T = hpool.tile([128, 128], bf16, tag="h1T")
            nc.vector.tensor_copy(h1T[:], tp1[:])
            # blkT[h, 128q + 32s + b] = h1T[h, 32q + b]
            nc.vector.tensor_copy(
                blkT[:].rearrange("p (q s b) -> p q s b", q=4, s=4)[:, :, s, :],
                h1T[:].rearrange("p (q b) -> p q b", q=4),
            )

        # ---- tail: projection for the last chunk ----
        for gi, (vlo, vhi) in enumerate(VCH):
            proj_group(blkT, vlo, vhi, 128 * (NCHUNK - 1),
                       "dve" if gi % 2 else "act")

        for p in (psx, psg, stp, bkp, ewpool, cpool, hpool, ppool,
                  wpool, consts):
            p.release()

    nc.compile()
    return nc


def _host_prep(sequence, enc_h, enc_c, emb, W_ih0, W_hh0, b_ih0, b_hh0,
               W_ih1, W_hh1, b_ih1, b_hh1, Wf, bf):
    bfl = ml_dtypes.bfloat16
    seq = np.asarray(sequence).astype(np.int64)
    emb = np.asarray(emb, np.float32)

    # seqG[32*s + b, c] = seq[b, 4*c + s]
    seqG = np.ascontiguousarray(
        seq.reshape(B, NCHUNK, 4).transpose(2, 0, 1).reshape(128, NCHUNK)
    ).astype(np.int32)

    WihT = np.asarray(W_ih0, np.float32).T        # [E+H, 4H]
    Wx = _rearrange_w_cols(np.ascontiguousarray(WihT[0:E]))
    Wfe = _rearrange_w_cols(np.ascontiguousarray(WihT[E:E + H]))
    Whh0 = _rearrange_w_cols(np.asarray(W_hh0, np.float32).T)
    Wih1 = _rearrange_w_cols(np.asarray(W_ih1, np.float32).T)
    Whh1 = _rearrange_w_cols(np.asarray(W_hh1, np.float32).T)

    # ptab = emb @ Wx + b0 (layer-0 x-part + bias, gate-rearranged cols)
    b0 = _rearrange_w_cols(
        (np.asarray(b_ih0, np.float32)
         + np.asarray(b_hh0, np.float32)).reshape(1, 4 * H))[0]
    ptab = (emb @ Wx + b0[None, :]).astype(bfl)

    def wtiles(w):
        return np.ascontiguousarray(w.reshape(4, 128, 4 * H)).astype(bfl)

    Wfp = np.zeros((VPAD, H), np.float32)
    Wfp[:V] = np.asarray(Wf, np.float32)

    identb = np.eye(128, dtype=np.float32).astype(bfl)
    id4 = np.tile(np.eye(32, dtype=np.float32), (4, 1)).astype(bfl)

    h0T = _hT_layout(np.asarray(enc_h[0], np.float32)).astype(bfl)
    h1T = _hT_layout(np.asarray(enc_h[1], np.float32)).astype(bfl)
    c0 = _x2_layout(np.asarray(enc_c[0], np.float32))
    c1 = _x2_layout(np.asarray(enc_c[1], np.float32))

    common = {
        "seqG": seqG,
        "ptab": ptab,
        "wfe": wtiles(Wfe), "whh0": wtiles(Whh0),
        "wih1": wtiles(Wih1), "whh1": wtiles(Whh1),
        "identb": identb, "id4": id4,
        "h0T": h0T, "h1T": h1T, "c0": c0, "c1": c1,
    }
    if not K_ZBIAS:
        common["b1g"] = _g_layout_bias(
            np.asarray(b_ih1, np.float32) + np.asarray(b_hh1, np.float32)
        ).astype(bfl)

    in_maps = []
    for cidx in range(NC_):
        m = dict(common)
        # wfT[q, h, v] = Wf[cidx*VSH + v, q*128 + h]
        shard = Wfp[cidx * VSH:(cidx + 1) * VSH]      # [VSH, H]
        m["wfT"] = np.ascontiguousarray(
            shard.T.reshape(4, 128, VSH)).astype(bfl)
        in_maps.append(m)
    return in_maps


last_results = None


def kernel(**inputs):
    from concourse.bass_utils import run_bass_kernel_spmd

    # layer-0 bias is folded into ptab; only layer-1 bias needs device work
    zb = all(
        not np.any(np.asarray(inputs[k]))
        for k in ("b_ih1", "b_hh1"))
    key = ("nc", zb)
    if key not in _cache:
        os.environ["K_ZBIAS"] = "1" if zb else "0"
        global K_ZBIAS
        K_ZBIAS = zb
        _cache[key] = _build_program()
    nc = _cache[key]

    in_maps = _host_prep(**inputs)
    trace = bool(int(os.environ.get("K_TRACE", "0")))
    res = run_bass_kernel_spmd(nc, in_maps, core_ids=list(range(NC_)),
                               trace=trace)
    global last_results
    last_results = res

    # assemble: logits [NTOK, VSH] bf16 per core, token = t*32 + b
    shards = []
    for c in range(NC_):
        lt = res.results[c]["logits"]                  # [4096, 6400] bf16
        shards.append(lt.reshape(S, B, VSH).transpose(1, 0, 2))
    full = np.concatenate(shards, axis=2)[:, :, :V].astype(np.float32)
    bfv = np.asarray(inputs["bf"], np.float32)
    if np.any(bfv):
        full = full + bfv[None, None, :]
    return np.ascontiguousarray(full)


# revision 20
# speedup vs baseline: 1.1214x; 1.1214x over previous
"""DecoderLSTM Trainium2 kernel v2 (8 NeuronCores, SPMD, no collectives).

Strategy (v2 — keeps the Tensor engine dense and warm):
  - LSTM recurrence replicated on all 8 cores (latency-bound, B=32).
  - Final projection sharded over vocab (VSH=6400 rows/core), computed
    TOKEN-STATIONARY: a [128-token x 512-hdim] block (4 steps of h1) is
    the PE stationary operand; Wf^T streams as the moving operand.
    Projection is interleaved per-step so it fills the PE-idle gaps the
    LSTM cell phases would otherwise leave (keeps HAM at K=8/8).
  - The x_t @ W_ih0[:, :E] (+ layer-0 bias) gate contribution does not
    depend on the recurrence OR the batch: ptab = emb @ W_ih0x + b0 is
    precomputed on the HOST ([V, 2048] bf16). The device just gathers
    128-token chunks of it (SWDGE indirect DMA) and injects them into
    each step's G0 PSUM accumulation with K=32 identity matmuls at
    tile_position rows.
  - Gates layout in PSUM: partition = 32*hq + b, free = g'*128 + hsub
    (gate order i,f,o,g), so elementwise ops use all 128 lanes.
  - All transposes are regular bf16 matmuls against an identity moving
    operand (faster than transpose-mode, and they keep HAM warm).
  - Output written as bf16 [NTOK, VSH] (halves the dominant HBM write);
    host reassembles/upcasts, and adds the vocab bias (usually zero).
"""

import os
import numpy as np
import ml_dtypes

K_ZBIAS = bool(int(os.environ.get("K_ZBIAS", "0")))

V, E, H, B, S = 50257, 512, 512, 32, 128
NC_ = 8
VSH = 6400                      # per-core padded vocab shard
VPAD = VSH * NC_                # 51200
NTOK = B * S                    # 4096 tokens, token = t*32 + b
NCHUNK = S // 4                 # 32 chunks of 4 steps = 128 tokens

# gate reorder: g' = (i, f, o, g) -> original pytorch order (i, f, g, o)
GPERM = (0, 1, 3, 2)

# vocab chunks for the projection moving operand (13 per 4-step group)
VCH = [(i * 512, min((i + 1) * 512, VSH)) for i in range((VSH + 511) // 512)]
PROJ_SCHED = {0: VCH[0:3], 1: VCH[3:6], 2: VCH[6:9], 3: VCH[9:13]}

_cache = {}


def _rearrange_w_cols(Wt):
    """Wt: [K, 4H] with original gate-column order (i,f,g,o) x H.
    Returns [K, 4H] with col' = hq*512 + g'*128 + hsub  mapping to
    original col = GPERM[g']*512 + hq*128 + hsub."""
    K = Wt.shape[0]
    w = Wt.reshape(K, 4, 4, 128)               # [K, g_orig, hq, hsub]
    out = np.empty((K, 4, 4, 128), Wt.dtype)   # [K, hq, g', hsub]
    for gp, go in enumerate(GPERM):
        out[:, :, gp, :] = w[:, go, :, :]
    return out.reshape(K, 4 * H)


def _g_layout_bias(bvec):
    """[4H] orig order -> [128, 512] G-layout tile (broadcast over b)."""
    r = _rearrange_w_cols(bvec.reshape(1, 4 * H))[0]   # col' order
    out = np.empty((128, 512), np.float32)
    for hq in range(4):
        out[32 * hq:32 * (hq + 1), :] = r[512 * hq:512 * (hq + 1)][None, :]
    return out


def _x2_layout(a):
    """[B, H] -> [128, 128] with partition 32*hq+b, free hsub."""
    return np.ascontiguousarray(
        a.reshape(B, 4, 128).transpose(1, 0, 2).reshape(128, 128))


def _hT_layout(a):
    """[B, H] -> [128, 128] with partition hsub, free hq*32+b."""
    return np.ascontiguousarray(
        a.reshape(B, 4, 128).transpose(2, 1, 0).reshape(128, 128))


def _build_program():
    import concourse.bass as bass
    import concourse.bacc as bacc
    import concourse.tile as tile
    from concourse import mybir

    f32 = mybir.dt.float32
    bf16 = mybir.dt.bfloat16
    i32 = mybir.dt.int32
    AF = mybir.ActivationFunctionType
    MUL = mybir.AluOpType.mult
    ADD = mybir.AluOpType.add

    nc = bacc.Bacc("TRN2", target_bir_lowering=False, debug=False,
                   enable_asserts=False, num_devices=NC_)

    d_seqG = nc.dram_tensor("seqG", [128, NCHUNK], i32, kind="ExternalInput").ap()
    d_ptab = nc.dram_tensor("ptab", [V, 4 * H], bf16, kind="ExternalInput").ap()
    d_wfe = nc.dram_tensor("wfe", [4, 128, 4 * H], bf16, kind="ExternalInput").ap()
    d_whh0 = nc.dram_tensor("whh0", [4, 128, 4 * H], bf16, kind="ExternalInput").ap()
    d_wih1 = nc.dram_tensor("wih1", [4, 128, 4 * H], bf16, kind="ExternalInput").ap()
    d_whh1 = nc.dram_tensor("whh1", [4, 128, 4 * H], bf16, kind="ExternalInput").ap()
    d_wfT = nc.dram_tensor("wfT", [4, 128, VSH], bf16, kind="ExternalInput").ap()
    d_identb = nc.dram_tensor("identb", [128, 128], bf16, kind="ExternalInput").ap()
    d_id4 = nc.dram_tensor("id4", [128, 32], bf16, kind="ExternalInput").ap()
    d_h0T = nc.dram_tensor("h0T", [128, 128], bf16, kind="ExternalInput").ap()
    d_h1T = nc.dram_tensor("h1T", [128, 128], bf16, kind="ExternalInput").ap()
    d_c0 = nc.dram_tensor("c0", [128, 128], f32, kind="ExternalInput").ap()
    d_c1 = nc.dram_tensor("c1", [128, 128], f32, kind="ExternalInput").ap()
    if not K_ZBIAS:
        d_b1g = nc.dram_tensor("b1g", [128, 512], bf16, kind="ExternalInput").ap()

    d_out = nc.dram_tensor("logits", [NTOK, VSH], bf16, kind="ExternalOutput").ap()

    with tile.TileContext(nc) as tc:
        consts = tc.alloc_tile_pool(name="consts", bufs=1)
        wpool = tc.alloc_tile_pool(name="weights", bufs=1)
        ppool = tc.alloc_tile_pool(name="pc", bufs=6)
        hpool = tc.alloc_tile_pool(name="hstate", bufs=3)
        cpool = tc.alloc_tile_pool(name="cstate", bufs=3)
        ewpool = tc.alloc_tile_pool(name="ew", bufs=3)
        bkp = tc.alloc_tile_pool(name="blk", bufs=2)
        stp = tc.alloc_tile_pool(name="stage", bufs=6)
        psg = tc.alloc_tile_pool(name="psg", bufs=3, space="PSUM")
        psx = tc.alloc_tile_pool(name="psx", bufs=5, space="PSUM")

        # ---- constants, initial state, first-needed weights ----
        identb = consts.tile([128, 128], bf16, tag="identb")
        nc.sync.dma_start(identb[:], d_identb[:])
        id4 = consts.tile([128, 32], bf16, tag="id4")
        nc.sync.dma_start(id4[:], d_id4[:])
        t_seqG = consts.tile([128, NCHUNK], i32, tag="seqG")
        nc.sync.dma_start(t_seqG[:], d_seqG[:])
        if not K_ZBIAS:
            t_b1g = consts.tile([128, 512], bf16, tag="b1g")
            nc.sync.dma_start(t_b1g[:], d_b1g[:])

        h0T = []
        h1T = []
        for q in range(4):
            h0q = hpool.tile([128, 32], bf16, tag=f"h0T{q}")
            nc.sync.dma_start(h0q[:], d_h0T[:, 32 * q:32 * (q + 1)])
            h0T.append(h0q)
            h1q = hpool.tile([128, 32], bf16, tag=f"h1T{q}")
            nc.sync.dma_start(h1q[:], d_h1T[:, 32 * q:32 * (q + 1)])
            h1T.append(h1q)
        c0 = cpool.tile([128, 128], f32, tag="c0")
        nc.sync.dma_start(c0[:], d_c0[:])
        c1 = cpool.tile([128, 128], f32, tag="c1")
        nc.sync.dma_start(c1[:], d_c1[:])

        def load_w(dram, name):
            ts = []
            for k in range(4):
                t = wpool.tile([128, 4 * H], bf16, tag=f"{name}{k}")
                nc.sync.dma_start(t[:], dram[k])
                ts.append(t)
            return ts

        # ---- helpers ----
        def gather_chunk(c):
            """gather 128 rows of ptab -> P chunk [128 tok, 2048] bf16"""
            pc = ppool.tile([128, 4 * H], bf16, tag="pc")
            nc.gpsimd.indirect_dma_start(
                out=pc[:], out_offset=None, in_=d_ptab[:],
                in_offset=bass.IndirectOffsetOnAxis(ap=t_seqG[:, c:c + 1], axis=0),
            )
            return pc

        # weight loads, ordered by first use: whh0/whh1/wih1 at step 0,
        # wfe at step 1, wfT from step 4 (loaded vocab-chunk-interleaved
        # so the first projection group unblocks after ~0.5 MB)
        whh0 = load_w(d_whh0, "whh0")
        pcs = {c: gather_chunk(c) for c in range(6)}
        whh1 = load_w(d_whh1, "whh1")
        wih1 = load_w(d_wih1, "wih1")
        wfe = load_w(d_wfe, "wfe")
        wfT = []
        for k in range(4):
            t = wpool.tile([128, VSH], bf16, tag=f"wfT{k}")
            wfT.append(t)
        for (vlo, vhi) in VCH:
            for k in range(4):
                nc.sync.dma_start(wfT[k][:, vlo:vhi], d_wfT[k, :, vlo:vhi])

        def emit_group(G, hTs, wts, first, last):
            """G += h-strips @ wts (K=512 as 4 k-tiles x 4 col-strips).
            hTs: list of 4 strip tiles [128, 32] (k-tile q of the
            contraction depends only on strip q)."""
            for k in range(4):
                lt = hTs[k][:]
                for cg in range(4):
                    nc.tensor.matmul(
                        G[32 * cg:32 * (cg + 1), :], lt,
                        wts[k][:, 512 * cg:512 * (cg + 1)],
                        start=(first and k == 0), stop=(last and k == 3),
                        tile_position=(0, 32 * cg), skip_group_check=True)

        def inject_p(G, pc, s, first):
            """G[32cg+m, n] (+)= pc[32s+m, 512cg+n] via K=32 identity MMs."""
            for cg in range(4):
                nc.tensor.matmul(
                    G[32 * cg:32 * (cg + 1), :],
                    id4[32 * s:32 * (s + 1), :],
                    pc[32 * s:32 * (s + 1), 512 * cg:512 * (cg + 1)],
                    start=first, stop=False,
                    tile_position=(32 * s, 32 * cg), skip_group_check=True)

        def inject_full(G, src, first):
            """G (+)= src ([128,512]) via K=128 identity MM."""
            nc.tensor.matmul(G[:], identb[:], src[:], start=first, stop=False,
                             skip_group_check=True)

        def cell(G, cprev, ctag):
            # sigmoid split into separate tiles: i/f gates feed the c-chain
            # immediately; the o gate is only needed for the final mult, so
            # it runs on ACT while the DVE works — dependency-clean because
            # each consumer reads its own tile.
            sif = ewpool.tile([128, 256], f32, tag="sif")
            nc.scalar.activation(sif[:], G[:, 0:256], AF.Sigmoid)
            tg = ewpool.tile([128, 128], f32, tag="tg")
            nc.scalar.activation(tg[:], G[:, 384:512], AF.Tanh)
            so = ewpool.tile([128, 128], f32, tag="so")
            nc.scalar.activation(so[:], G[:, 256:384], AF.Sigmoid)
            m2 = ewpool.tile([128, 128], f32, tag="m2")
            nc.vector.tensor_tensor(m2[:], sif[:, 128:256], cprev[:], op=MUL)
            m1 = ewpool.tile([128, 128], f32, tag="m1")
            nc.vector.tensor_tensor(m1[:], sif[:, 0:128], tg[:], op=MUL)
            cn = cpool.tile([128, 128], f32, tag=ctag)
            nc.vector.tensor_tensor(cn[:], m1[:], m2[:], op=ADD)
            tc_ = ewpool.tile([128, 128], f32, tag="tc")
            nc.scalar.activation(tc_[:], cn[:], AF.Tanh)
            hx = ewpool.tile([128, 128], bf16, tag="hx")
            nc.vector.tensor_tensor(hx[:], so[:], tc_[:], op=MUL)
            return hx, cn

        def transpose_mm(hx):
            tp = psx.tile([128, 128], f32, tag="ps")
            nc.tensor.matmul(tp[:], hx[:], identb[:], start=True, stop=True)
            return tp

        def cast_strips(tp, tagbase):
            """PSUM transposed h -> 4 bf16 strip tiles [128, 32]; the next
            matmul group's k-tile q depends only on strip q, so k0 can
            start after 1/4 of the cast work."""
            strips = []
            for q in range(4):
                hq = hpool.tile([128, 32], bf16, tag=f"{tagbase}{q}")
                nc.vector.tensor_copy(hq[:], tp[:, 32 * q:32 * (q + 1)])
                strips.append(hq)
            return strips

        def proj_group(bt, vlo, vhi, row0, eng="act"):
            n = vhi - vlo
            pj = psx.tile([128, 512], f32, tag="ps")
            for q in range(4):
                nc.tensor.matmul(pj[:, 0:n], bt[:, 128 * q:128 * (q + 1)],
                                 wfT[q][:, vlo:vhi],
                                 start=(q == 0), stop=(q == 3))
            st = stp.tile([128, 512], bf16, tag="st")
            if eng == "act":
                nc.scalar.copy(st[:, 0:n], pj[:, 0:n])
            else:
                nc.vector.tensor_copy(st[:, 0:n], pj[:, 0:n])
            nc.sync.dma_start(d_out[row0:row0 + 128, vlo:vhi], st[:, 0:n])

        # ---- main loop ----
        G0 = G1 = G0n = None
        blkT = blkT_prev = None

        for t in range(S):
            c, s = divmod(t, 4)
            if s == 0:
                blkT_prev, blkT = blkT, bkp.tile([128, 512], bf16, tag="blkT")
                if c + 6 < NCHUNK:
                    pcs[c + 6] = gather_chunk(c + 6)

            # (a) close G0(t): feed group (skipped at t=0: input_feed is 0)
            if t == 0:
                G0 = psg.tile([128, 512], f32, tag="G")
                inject_p(G0, pcs[0], 0, first=True)
                emit_group(G0, h0T, whh0, first=False, last=True)
            else:
                G0 = G0n
                emit_group(G0, h1T, wfe, first=False, last=True)

            # (b) cell0
            h0x, c0 = cell(G0, c0, "c0")

            # (c) prestart G1(t): h1prev part (+ bias) — fills cell0 gap
            G1 = psg.tile([128, 512], f32, tag="G")
            if not K_ZBIAS:
                inject_full(G1, t_b1g, first=True)
                emit_group(G1, h1T, whh1, first=False, last=False)
            else:
                emit_group(G1, h1T, whh1, first=True, last=False)

            # (d) projection fillers (chunk c-1); ACT copies queue after
            # cell0's activations so they never delay the cell chain
            if c >= 1:
                for (vlo, vhi) in PROJ_SCHED[s][:2]:
                    proj_group(blkT_prev, vlo, vhi, 128 * (c - 1))

            # (e) transpose h0
            tp0 = transpose_mm(h0x)
            h0T = cast_strips(tp0, "h0T")

            # (f) close G1(t): h0 group
            emit_group(G1, h0T, wih1, first=False, last=True)

            # (g) cell1
            h1x, c1 = cell(G1, c1, "c1")

            # (h) prestart G0(t+1): P inject + h0prev — fills cell1 gap
            if t + 1 < S:
                cn_, sn = divmod(t + 1, 4)
                G0n = psg.tile([128, 512], f32, tag="G")
                inject_p(G0n, pcs[cn_], sn, first=True)
                emit_group(G0n, h0T, whh0, first=False, last=False)

            # (i) remaining projection fillers
            if c >= 1:
                rest = PROJ_SCHED[s][2:]
                for gi, (vlo, vhi) in enumerate(rest):
                    eng = "dve" if (s == 3 and gi == len(rest) - 1) else "act"
                    proj_group(blkT_prev, vlo, vhi, 128 * (c - 1), eng)

            # (j) transpose h1 -> h1T strips + blkT columns
            tp1 = transpose_mm(h1x)
            h1T = cast_strips(tp1, "h1T")
            # blkT[h, 128q + 32s + b] = h1(t)[b, 128q + h]
            for q in range(4):
                nc.vector.tensor_copy(
                    blkT[:, 128 * q + 32 * s:128 * q + 32 * s + 32],
                    h1T[q][:])

        # ---- tail: projection for the last chunk ----
        for gi, (vlo, vhi) in enumerate(VCH):
            proj_group(blkT, vlo, vhi, 128 * (NCHUNK - 1),
                       "dve" if gi % 2 else "act")

        for p in (psx, psg, stp, bkp, ewpool, cpool, hpool, ppool,
                  wpool, consts):
            p.release()

    nc.compile()
    return nc


def _host_prep(sequence, enc_h, enc_c, emb, W_ih0, W_hh0, b_ih0, b_hh0,
               W_ih1, W_hh1, b_ih1, b_hh1, Wf, bf):
    bfl = ml_dtypes.bfloat16
    seq = np.asarray(sequence).astype(np.int64)
    emb = np.asarray(emb, np.float32)

    # seqG[32*s + b, c] = seq[b, 4*c + s]
    seqG = np.ascontiguousarray(
        seq.reshape(B, NCHUNK, 4).transpose(2, 0, 1).reshape(128, NCHUNK)
    ).astype(np.int32)

    WihT = np.asarray(W_ih0, np.float32).T        # [E+H, 4H]
    Wx = _rearrange_w_cols(np.ascontiguousarray(WihT[0:E]))
    Wfe = _rearrange_w_cols(np.ascontiguousarray(WihT[E:E + H]))
    Whh0 = _rearrange_w_cols(np.asarray(W_hh0, np.float32).T)
    Wih1 = _rearrange_w_cols(np.asarray(W_ih1, np.float32).T)
    Whh1 = _rearrange_w_cols(np.asarray(W_hh1, np.float32).T)

    # ptab = emb @ Wx + b0 (layer-0 x-part + bias, gate-rearranged cols)
    b0 = _rearrange_w_cols(
        (np.asarray(b_ih0, np.float32)
         + np.asarray(b_hh0, np.float32)).reshape(1, 4 * H))[0]
    ptab = (emb @ Wx + b0[None, :]).astype(bfl)

    def wtiles(w):
        return np.ascontiguousarray(w.reshape(4, 128, 4 * H)).astype(bfl)

    Wfp = np.zeros((VPAD, H), np.float32)
    Wfp[:V] = np.asarray(Wf, np.float32)

    identb = np.eye(128, dtype=np.float32).astype(bfl)
    id4 = np.tile(np.eye(32, dtype=np.float32), (4, 1)).astype(bfl)

    h0T = _hT_layout(np.asarray(enc_h[0], np.float32)).astype(bfl)
    h1T = _hT_layout(np.asarray(enc_h[1], np.float32)).astype(bfl)
    c0 = _x2_layout(np.asarray(enc_c[0], np.float32))
    c1 = _x2_layout(np.asarray(enc_c[1], np.float32))

    common = {
        "seqG": seqG,
        "ptab": ptab,
        "wfe": wtiles(Wfe), "whh0": wtiles(Whh0),
        "wih1": wtiles(Wih1), "whh1": wtiles(Whh1),
        "identb": identb, "id4": id4,
        "h0T": h0T, "h1T": h1T, "c0": c0, "c1": c1,
    }
    if not K_ZBIAS:
        common["b1g"] = _g_layout_bias(
            np.asarray(b_ih1, np.float32) + np.asarray(b_hh1, np.float32)
        ).astype(bfl)

    in_maps = []
    for cidx in range(NC_):
        m = dict(common)
        # wfT[q, h, v] = Wf[cidx*VSH + v, q*128 + h]
        shard = Wfp[cidx * VSH:(cidx + 1) * VSH]      # [VSH, H]
        m["wfT"] = np.ascontiguousarray(
            shard.T.reshape(4, 128, VSH)).astype(bfl)
        in_maps.append(m)
    return in_maps


last_results = None


def kernel(**inputs):
    from concourse.bass_utils import run_bass_kernel_spmd

    # layer-0 bias is folded into ptab; only layer-1 bias needs device work
    zb = all(
        not np.any(np.asarray(inputs[k]))
        for k in ("b_ih1", "b_hh1"))
    key = ("nc", zb)
    if key not in _cache:
        os.environ["K_ZBIAS"] = "1" if zb else "0"
        global K_ZBIAS
        K_ZBIAS = zb
        _cache[key] = _build_program()
    nc = _cache[key]

    in_maps = _host_prep(**inputs)
    trace = bool(int(os.environ.get("K_TRACE", "0")))
    res = run_bass_kernel_spmd(nc, in_maps, core_ids=list(range(NC_)),
                               trace=trace)
    global last_results
    last_results = res

    # assemble: logits [NTOK, VSH] bf16 per core, token = t*32 + b
    shards = []
    for c in range(NC_):
        lt = res.results[c]["logits"]                  # [4096, 6400] bf16
        shards.append(lt.reshape(S, B, VSH).transpose(1, 0, 2))
    full = np.concatenate(shards, axis=2)[:, :, :V].astype(np.float32)
    bfv = np.asarray(inputs["bf"], np.float32)
    if np.any(bfv):
        full = full + bfv[None, None, :]
    return np.ascontiguousarray(full)
